# revision 10
# baseline (speedup 1.0000x reference)
"""ACAR head (grouped ROI attention) Trainium2 Bass kernel.

Strategy: data-parallel over ROI groups. roi_inds has NUM_CLIPS=8 groups and
there are 8 NeuronCores, so core c owns group c (padded to a common Npad).
Attention never crosses groups, so there is no inter-core communication; the
host shards inputs / gathers outputs.

Compute dtype: float32r (TF32-like rounded fp32) — full-rate on the PE at
free-dim >= 256, ~1.5e-4 relative rounding.
"""

import os
import sys
import types

sys.path.insert(0, "/opt/trn_rl_repo")

import numpy as np
import ml_dtypes


def _install_ntff_hook():
    """The image's antenv package lacks axon_hooks; inject it so trace=True
    can capture NTFF profiles. Harmless if anything is missing."""
    try:
        import antenv  # noqa: F401
        from trn_agent_boot.trn_boot import _ntff_profile_via_ctypes

        hook = _ntff_profile_via_ctypes("/opt/axon/libaxon_pjrt.so")
        if hook is None:
            return False
        mod = types.ModuleType("antenv.axon_hooks")
        mod.get_axon_ntff_profile_hook = lambda: hook
        mod.set_axon_ntff_profile_hook = lambda h: None
        sys.modules["antenv.axon_hooks"] = mod
        return True
    except Exception:
        return False


import concourse.bass as bass
import concourse.bacc as bacc
import concourse.tile as tile
from concourse import mybir
from concourse.bass_utils import run_bass_kernel_spmd
from concourse.masks import make_identity

F32 = mybir.dt.float32
F32R = mybir.dt.float32r
BF16 = mybir.dt.bfloat16

N_CORES = 8
N, C, T, H, W = 256, 512, 4, 8, 8
HW = H * W
CC = C // 128          # c chunks
NA = 64                # attention row pad (group size must be <= 64)
GN_EPS = 1e-5

LAST_EXEC_NS = None


def _build(npad: int):
    nbk = npad // 8
    nc = bacc.Bacc("TRN2", target_bir_lowering=False, debug=False,
                   num_devices=N_CORES)

    # ---- dram parameters (per-core shards) ----
    xp_d = nc.dram_tensor("xp", [CC, 128, npad // 8, T, 100, 8], BF16,
                          kind="ExternalInput").ap()
    xr_d = nc.dram_tensor("xr", [CC, 128, T, npad // 8, 512], F32,
                          kind="ExternalInput").ap()
    wq_d = nc.dram_tensor("wq", [CC, 128, 9, C], BF16, kind="ExternalInput").ap()
    wk_d = nc.dram_tensor("wk", [CC, 128, 9, C], BF16, kind="ExternalInput").ap()
    wv_d = nc.dram_tensor("wv", [CC, 128, 9, C], BF16, kind="ExternalInput").ap()
    wc_d = nc.dram_tensor("wc", [CC, 128, 9, C], BF16, kind="ExternalInput").ap()
    mask_d = nc.dram_tensor("mask", [NA], F32, kind="ExternalInput").ap()
    gamma_d = nc.dram_tensor("gamma", [C], F32, kind="ExternalInput").ap()
    beta_d = nc.dram_tensor("beta", [C], F32, kind="ExternalInput").ap()
    out_d = nc.dram_tensor("out", [CC, 128, T, npad // 8, 512], F32,
                           kind="ExternalOutput").ap()

    def bcast_ap(src, n_part, extra):
        return bass.AP(tensor=src.tensor, offset=src.offset,
                       ap=[[0, n_part]] + extra)

    with tile.TileContext(nc) as tc:
        with (
            tc.tile_pool(name="singles", bufs=1) as singles,
            tc.tile_pool(name="dram", bufs=1, space="DRAM") as dpool,
        ):
            ident = singles.tile([128, 128], F32)
            make_identity(nc, ident)
            ident_bf = singles.tile([128, 128], BF16)
            nc.vector.tensor_copy(out=ident_bf, in_=ident)
            mask_b = singles.tile([128, NA], F32)
            nc.gpsimd.dma_start(out=mask_b,
                                in_=bcast_ap(mask_d, 128, [[1, NA]]))
            zeros1 = singles.tile([128, 1], F32)
            nc.vector.memset(zeros1, 0.0)
            eps_t = singles.tile([64, 1], F32)
            nc.vector.memset(eps_t, GN_EPS)
            gam = singles.tile([128, CC], F32)
            bet = singles.tile([128, CC], F32)
            for cc in range(CC):
                nc.sync.dma_start(out=gam[:, cc:cc + 1],
                                  in_=gamma_d[cc * 128:(cc + 1) * 128])
                nc.sync.dma_start(out=bet[:, cc:cc + 1],
                                  in_=beta_d[cc * 128:(cc + 1) * 128])
            # per-(i, pair) bn stats: partitions 0:64 = even locs, 64:128 = odd
            stats = singles.tile([128, 128, 6], F32)

            vsp = dpool.tile([T, 32, 128, C], F32)
            mvd = dpool.tile([128, 2], F32)
            gnd = dpool.tile([64, 2], F32)

            # ============ Fused conv(q,k,v) + attention, per t ============
            with (
                tc.tile_pool(name="wA", bufs=1) as wpool,
                tc.tile_pool(name="xA", bufs=1) as xpool,
                tc.tile_pool(name="qkvB", bufs=1) as qkv,
                tc.tile_pool(name="sbB", bufs=2) as pB,
                tc.tile_pool(name="psAB", bufs=1, space="PSUM") as psum,
            ):
                for t in range(T):
                    # x tiles for this t (shared by all three convs)
                    xs = {}
                    for nb in range(nbk):
                        for cc in range(CC):
                            xt = xpool.tile([128, 100, 8], BF16,
                                            tag=f"x{nb}_{cc}",
                                            name=f"x{nb}_{cc}")
                            nc.sync.dma_start(out=xt, in_=xp_d[cc, :, nb, t])
                            xs[(nb, cc)] = xt
                    # qkv half tiles [c, 32hw, 64n], zero pad rows
                    qkv_sb = {}
                    for name, wd in (("q", wq_d), ("k", wk_d), ("v", wv_d)):
                        w_sb = []
                        for cc in range(CC):
                            wt = wpool.tile([128, 9, C], BF16,
                                            tag=f"w{cc}", name=f"w{cc}")
                            nc.sync.dma_start(out=wt, in_=wd[cc])
                            w_sb.append(wt)
                        for half in range(2):
                            for cc in range(CC):
                                tl = qkv.tile([128, 32, NA], BF16,
                                              tag=f"{name}{half}{cc}",
                                              name=f"{name}{half}{cc}")
                                if npad < NA:
                                    nc.vector.tensor_copy(
                                        out=tl[:, :, npad:NA],
                                        in_=bass.AP(
                                            tensor=zeros1.tensor,
                                            offset=zeros1.offset,
                                            ap=[zeros1.ap[0], [0, 32],
                                                [0, NA - npad]]),
                                    )
                                qkv_sb[(name, half, cc)] = tl
                        for nb in range(nbk):
                            for oc in range(4):
                                ps = psum.tile([128, 512], F32, tag="cps",
                                               bufs=2)
                                for cc in range(CC):
                                    for s in range(9):
                                        dh, dw = s // 3, s % 3
                                        xt = xs[(nb, cc)]
                                        rhs = bass.AP(
                                            tensor=xt.tensor,
                                            offset=xt.offset
                                            + (dh * 10 + dw) * 8,
                                            ap=[xt.ap[0], [80, 8], [8, 8],
                                                [1, 8]],
                                        )
                                        nc.tensor.matmul(
                                            ps,
                                            lhsT=w_sb[cc][:, s,
                                                          oc * 128:(oc + 1) * 128],
                                            rhs=rhs,
                                            start=(cc == 0 and s == 0),
                                            stop=(cc == CC - 1 and s == 8),
                                        )
                                # copy PSUM (h,w,n) into the half tiles
                                for half in range(2):
                                    tl = qkv_sb[(name, half, oc)]
                                    src_ap = bass.AP(
                                        tensor=ps.tensor,
                                        offset=ps.offset + half * 32 * 8,
                                        ap=[ps.ap[0], [8, 32], [1, 8]])
                                    dst_ap = bass.AP(
                                        tensor=tl.tensor,
                                        offset=tl.offset + nb * 8,
                                        ap=[tl.ap[0], [NA, 32], [1, 8]])
                                    nc.vector.tensor_copy(out=dst_ap,
                                                          in_=src_ap)
                    # ---- attention for this t ----
                    for half in range(2):
                        q_sb = [qkv_sb[("q", half, cc)] for cc in range(CC)]
                        k_sb = [qkv_sb[("k", half, cc)] for cc in range(CC)]
                        v_sb = [qkv_sb[("v", half, cc)] for cc in range(CC)]
                        for quad in range(8):
                            h4 = quad * 4
                            sp = psum.tile([128, 512], F32, tag="s", bufs=2)
                            for sub in range(2):
                                for cc in range(CC):
                                    nc.tensor.matmul(
                                        sp[:, 256 * sub:256 * sub + 256],
                                        lhsT=q_sb[cc][:, h4 + 2 * sub:
                                                      h4 + 2 * sub + 2, :],
                                        rhs=k_sb[cc][:, h4:h4 + 4, :],
                                        start=(cc == 0), stop=(cc == CC - 1),
                                    )
                            for pp in range(2):
                                s_ps = sp
                                pair = t * 32 + half * 16 + quad * 2 + pp
                                e_sb = pB.tile([128, 128], BF16, tag="e")
                                nc.vector.memset(e_sb[0:64, 64:128], 0.0)
                                nc.vector.memset(e_sb[64:128, 0:64], 0.0)
                                nm = pB.tile([128, 1], F32, tag="nm")
                                dsum = pB.tile([128, 1], F32, tag="d")
                                rr = pB.tile([128, 1], F32, tag="r")
                                for l in range(2):
                                    rs = slice(64 * l, 64 * l + 64)
                                    cs = slice(384 * pp + 64 * l,
                                               384 * pp + 64 * l + 64)
                                    sm = pB.tile([128, 64], F32, tag="sm")
                                    nc.vector.tensor_tensor(
                                        out=sm[rs], in0=s_ps[rs, cs],
                                        in1=mask_b[rs],
                                        op=mybir.AluOpType.add)
                                    nc.vector.tensor_reduce(
                                        out=nm[rs], in_=sm[rs],
                                        axis=mybir.AxisListType.X,
                                        op=mybir.AluOpType.max, negate=True)
                                    nc.scalar.activation(
                                        out=e_sb[rs, rs], in_=sm[rs],
                                        func=mybir.ActivationFunctionType.Exp,
                                        bias=nm[rs], scale=1.0,
                                        accum_out=dsum[rs])
                                nc.vector.reciprocal(out=rr, in_=dsum)

                                et_ps = psum.tile([128, 128], BF16,
                                                  tag="et_ps", bufs=1)
                                nc.tensor.transpose(et_ps, e_sb, ident_bf)
                                et = pB.tile([128, 128], BF16, tag="et")
                                nc.vector.tensor_copy(out=et, in_=et_ps)

                                vpair = pB.tile([128, C], BF16, tag="vp")
                                for cc in range(CC):
                                    vt_ps = psum.tile([128, 128], BF16,
                                                      tag="vt_ps", bufs=2)
                                    nc.tensor.transpose(
                                        vt_ps,
                                        v_sb[cc][:, h4 + 2 * pp:
                                                 h4 + 2 * pp + 2, :],
                                        ident_bf)
                                    nc.vector.tensor_copy(
                                        out=vpair[:, cc * 128:(cc + 1) * 128],
                                        in_=vt_ps)

                                av_ps = psum.tile([128, C], F32, tag="av",
                                                  bufs=1)
                                nc.tensor.matmul(av_ps, lhsT=et, rhs=vpair,
                                                 start=True, stop=True)
                                vb = pB.tile([128, C], F32, tag="vb")
                                nc.vector.tensor_scalar_mul(vb, av_ps, rr)
                                nc.vector.bn_stats(out=stats[:, pair, :],
                                                   in_=vb)
                                nc.sync.dma_start(
                                    out=vsp[t, half * 16 + quad * 2 + pp],
                                    in_=vb)

                # ---- GroupNorm stats finalize ----
                mv = pB.tile([128, 2], F32, tag="mv")
                nc.vector.bn_aggr(out=mv, in_=stats)
                nc.sync.dma_start(out=mvd, in_=mv)
                mva = pB.tile([64, 2], F32, tag="mva")
                mvb = pB.tile([64, 2], F32, tag="mvb")
                nc.sync.dma_start(out=mva, in_=mvd[0:64])
                nc.sync.dma_start(out=mvb, in_=mvd[64:128])
                mu = pB.tile([64, 1], F32, tag="mu")
                nc.vector.tensor_add(mu, mva[:, 0:1], mvb[:, 0:1])
                nc.vector.tensor_scalar_mul(mu, mu, 0.5)
                dm = pB.tile([64, 1], F32, tag="dm")
                nc.vector.tensor_sub(dm, mva[:, 0:1], mvb[:, 0:1])
                nc.vector.tensor_scalar_mul(dm, dm, 0.5)
                nc.vector.tensor_mul(dm, dm, dm)
                var = pB.tile([64, 1], F32, tag="var")
                nc.vector.tensor_add(var, mva[:, 1:2], mvb[:, 1:2])
                nc.vector.tensor_scalar_mul(var, var, 0.5)
                nc.vector.tensor_add(var, var, dm)
                rstd = pB.tile([64, 1], F32, tag="rstd")
                nc.scalar.activation(out=rstd, in_=var,
                                     func=mybir.ActivationFunctionType.Sqrt,
                                     bias=eps_t, scale=1.0)
                nc.vector.reciprocal(out=rstd, in_=rstd)
                murstd = pB.tile([64, 1], F32, tag="murstd")
                nc.vector.tensor_mul(murstd, mu, rstd)
                gpack = pB.tile([64, 2], F32, tag="gpack")
                nc.vector.tensor_copy(out=gpack[:, 0:1], in_=rstd)
                nc.vector.tensor_copy(out=gpack[:, 1:2], in_=murstd)
                nc.sync.dma_start(out=gnd, in_=gpack)

            # broadcast (rstd, mu*rstd) along partitions: [128, 64, 2]
            abn = singles.tile([128, 64, 2], F32)
            nc.gpsimd.dma_start(out=abn,
                                in_=bcast_ap(gnd, 128, [[2, 64], [1, 2]]))

            # ================= Phase C: GN apply + Wc conv + residual ====
            with (
                tc.tile_pool(name="wC", bufs=1) as wpool,
                tc.tile_pool(name="vtC", bufs=3) as vtpool,
                tc.tile_pool(name="padC", bufs=1) as padpool,
                tc.tile_pool(name="ioC", bufs=3) as iopool,
                tc.tile_pool(name="psC", bufs=1, space="PSUM") as psC,
            ):
                wc_sb = []
                for cc in range(CC):
                    wt = wpool.tile([128, 9, C], BF16, tag=f"wc{cc}")
                    nc.sync.dma_start(out=wt, in_=wc_d[cc])
                    wc_sb.append(wt)
                # persistent pre-zeroed padded tiles (double-buffered manually)
                vpads = []
                for par in range(2):
                    row = []
                    for cc in range(CC):
                        vp = padpool.tile([128, 100, 8], BF16,
                                          tag=f"vp{par}_{cc}")
                        nc.vector.tensor_copy(
                            out=vp,
                            in_=bass.AP(tensor=zeros1.tensor,
                                        offset=zeros1.offset,
                                        ap=[zeros1.ap[0], [0, 100], [0, 8]]))
                        row.append(vp)
                    vpads.append(row)
                it_c = 0
                for t in range(T):
                    vt_big = [vtpool.tile([128, 32, 2, NA], BF16,
                                          tag=f"vt{cc}", name=f"vt{cc}")
                              for cc in range(CC)]
                    for pr in range(32):
                        vb_r = iopool.tile([128, C], F32, tag="vbr",
                                           bufs=6)
                        nc.scalar.dma_start(out=vb_r, in_=vsp[t, pr])
                        for cc in range(CC):
                            tp = psC.tile([128, 128], F32, tag="tp", bufs=2)
                            nc.tensor.transpose(
                                tp, vb_r[:, cc * 128:(cc + 1) * 128], ident)
                            nc.vector.tensor_copy(out=vt_big[cc][:, pr, :, :],
                                                  in_=tp)
                    for cc in range(CC):
                        vt = vt_big[cc]
                        rstd_b = bass.AP(tensor=abn.tensor, offset=abn.offset,
                                         ap=[abn.ap[0], [0, 32], [0, 2],
                                             [2, NA]])
                        murstd_b = bass.AP(tensor=abn.tensor,
                                           offset=abn.offset + 1,
                                           ap=[abn.ap[0], [0, 32], [0, 2],
                                               [2, NA]])
                        nc.vector.tensor_tensor(out=vt, in0=vt, in1=rstd_b,
                                                op=mybir.AluOpType.mult)
                        nc.vector.tensor_tensor(out=vt, in0=vt, in1=murstd_b,
                                                op=mybir.AluOpType.subtract)
                        nc.vector.tensor_scalar(
                            out=vt, in0=vt, scalar1=gam[:, cc:cc + 1],
                            scalar2=bet[:, cc:cc + 1],
                            op0=mybir.AluOpType.mult,
                            op1=mybir.AluOpType.add)
                        nc.scalar.activation(
                            out=vt, in_=vt,
                            func=mybir.ActivationFunctionType.Relu)
                    for nb in range(nbk):
                        par = it_c % 2
                        it_c += 1
                        for cc in range(CC):
                            src = bass.AP(
                                tensor=vt_big[cc].tensor,
                                offset=vt_big[cc].offset + nb * 8,
                                ap=[vt_big[cc].ap[0], [512, 8], [64, 8],
                                    [1, 8]])
                            dst = bass.AP(
                                tensor=vpads[par][cc].tensor,
                                offset=vpads[par][cc].offset + 11 * 8,
                                ap=[vpads[par][cc].ap[0], [80, 8], [8, 8],
                                    [1, 8]])
                            nc.vector.tensor_copy(out=dst, in_=src)
                        for oc in range(4):
                            ps = psC.tile([128, 512], F32, tag="cps", bufs=5)
                            for cc in range(CC):
                                for s in range(9):
                                    dh, dw = s // 3, s % 3
                                    vp = vpads[par][cc]
                                    rhs = bass.AP(
                                        tensor=vp.tensor,
                                        offset=vp.offset + (dh * 10 + dw) * 8,
                                        ap=[vp.ap[0], [80, 8], [8, 8],
                                            [1, 8]])
                                    nc.tensor.matmul(
                                        ps,
                                        lhsT=wc_sb[cc][:, s,
                                                       oc * 128:(oc + 1) * 128],
                                        rhs=rhs,
                                        start=(cc == 0 and s == 0),
                                        stop=(cc == CC - 1 and s == 8))
                            xr = iopool.tile([128, 512], F32, tag="xr")
                            nc.sync.dma_start(out=xr,
                                              in_=xr_d[oc, :, t, nb])
                            ob = iopool.tile([128, 512], F32, tag="cob")
                            nc.vector.tensor_add(ob, ps, xr)
                            nc.sync.dma_start(out=out_d[oc, :, t, nb],
                                               in_=ob)

    nc.compile()
    return nc


_BUILD_CACHE = {}


def kernel(x, roi_inds, Wq, Wk, Wv, Wc, gn_gamma, gn_beta):
    global LAST_EXEC_NS
    x = np.ascontiguousarray(np.asarray(x, dtype=np.float32))
    roi_inds = np.asarray(roi_inds, dtype=np.int32)
    n, c = x.shape[0], x.shape[1]
    assert (n, c) == (N, C) and x.shape[2:] == (T, H, W)

    # group ROIs per clip; core g <- group g
    order = np.argsort(roi_inds, kind="stable")
    groups = [order[roi_inds[order] == g] for g in range(N_CORES)]
    sizes = [len(g) for g in groups]
    max_sz = max(sizes)
    assert max_sz <= NA, f"group size {max_sz} > {NA} unsupported"
    npad = ((max_sz + 7) // 8) * 8

    scale = 1.0 / np.sqrt(np.float32(C))

    def prep_w(Wt, sc=1.0):
        # [O, C, 1, 3, 3] -> [CC, 128, 9, O]
        w = (np.asarray(Wt, np.float32)[:, :, 0] * sc)  # [O, C, 3, 3]
        w = w.transpose(1, 2, 3, 0).reshape(CC, 128, 9, C)
        return np.ascontiguousarray(w).astype(ml_dtypes.bfloat16)

    w_arrs = {
        "wq": prep_w(Wq, scale), "wk": prep_w(Wk), "wv": prep_w(Wv),
        "wc": prep_w(Wc),
    }
    gamma = np.ascontiguousarray(np.asarray(gn_gamma, np.float32))
    beta = np.ascontiguousarray(np.asarray(gn_beta, np.float32))

    in_maps = []
    for g in range(N_CORES):
        idx = groups[g]
        xg = np.zeros((npad, C, T, H, W), np.float32)
        xg[:sizes[g]] = x[idx]
        # xp: [CC, 128, npad, T, 10, 10] zero-padded -> [CC,128,npad,T,100]
        xcm = xg.transpose(1, 0, 2, 3, 4).reshape(CC, 128, npad, T, H, W)
        xpp = np.zeros((CC, 128, npad, T, 10, 10), np.float32)
        xpp[..., 1:9, 1:9] = xcm
        # -> [CC, 128, NBK, T, 100, 8] with n innermost (contiguous for PE)
        xpp = xpp.reshape(CC, 128, npad // 8, 8, T, 100)
        xp = np.ascontiguousarray(
            xpp.transpose(0, 1, 2, 4, 5, 3)).astype(ml_dtypes.bfloat16)
        # xr: [CC, 128, T, NBK, (h,w,n8)] matching conv PSUM column order
        xr = np.ascontiguousarray(
            xcm.reshape(CC, 128, npad // 8, 8, T, HW)
            .transpose(0, 1, 4, 2, 5, 3)).reshape(
                CC, 128, T, npad // 8, 512)
        mask = np.zeros(NA, np.float32)
        mask[sizes[g]:] = -1e30
        m = {"xp": xp, "xr": xr, "mask": mask,
             "gamma": gamma, "beta": beta}
        m.update(w_arrs)
        in_maps.append(m)

    if npad not in _BUILD_CACHE:
        _BUILD_CACHE[npad] = _build(npad)
    nc = _BUILD_CACHE[npad]

    trace = bool(int(os.environ.get("ACAR_TRACE", "0")))
    if trace:
        _install_ntff_hook()
    res = run_bass_kernel_spmd(nc, in_maps, core_ids=list(range(N_CORES)),
                               trace=trace)
    LAST_EXEC_NS = res.exec_time_ns

    out = np.empty((N, C, T, H, W), np.float32)
    for g in range(N_CORES):
        og = res.results[g]["out"]  # [CC, 128, T, NBK, (hw, n8)]
        og = og.reshape(C, T, npad // 8, HW, 8)
        og = og.transpose(2, 4, 0, 1, 3).reshape(npad, C, T, H, W)
        out[groups[g]] = og[:sizes[g]]
    return out


# revision 11
# speedup vs baseline: 1.1959x; 1.1959x over previous
"""ACAR head (grouped ROI attention) Trainium2 Bass kernel.

Strategy: data-parallel over ROI groups. roi_inds has NUM_CLIPS=8 groups and
there are 8 NeuronCores, so core c owns group c (padded to a common Npad).
Attention never crosses groups, so there is no inter-core communication; the
host shards inputs / gathers outputs.

Compute dtype: float32r (TF32-like rounded fp32) — full-rate on the PE at
free-dim >= 256, ~1.5e-4 relative rounding.
"""

import os
import sys
import types

sys.path.insert(0, "/opt/trn_rl_repo")

import numpy as np
import ml_dtypes


def _install_ntff_hook():
    """The image's antenv package lacks axon_hooks; inject it so trace=True
    can capture NTFF profiles. Harmless if anything is missing."""
    try:
        import antenv  # noqa: F401
        from trn_agent_boot.trn_boot import _ntff_profile_via_ctypes

        hook = _ntff_profile_via_ctypes("/opt/axon/libaxon_pjrt.so")
        if hook is None:
            return False
        mod = types.ModuleType("antenv.axon_hooks")
        mod.get_axon_ntff_profile_hook = lambda: hook
        mod.set_axon_ntff_profile_hook = lambda h: None
        sys.modules["antenv.axon_hooks"] = mod
        return True
    except Exception:
        return False


import concourse.bass as bass
import concourse.bacc as bacc
import concourse.tile as tile
from concourse import mybir
from concourse.bass_utils import run_bass_kernel_spmd
from concourse.masks import make_identity

F32 = mybir.dt.float32
F32R = mybir.dt.float32r
BF16 = mybir.dt.bfloat16

N_CORES = 8
N, C, T, H, W = 256, 512, 4, 8, 8
HW = H * W
CC = C // 128          # c chunks
NA = 64                # attention row pad (group size must be <= 64)
GN_EPS = 1e-5

LAST_EXEC_NS = None


def _build(npad: int):
    nbk = npad // 8
    nc = bacc.Bacc("TRN2", target_bir_lowering=False, debug=False,
                   num_devices=N_CORES)

    # ---- dram parameters (per-core shards) ----
    xp_d = nc.dram_tensor("xp", [CC, 128, npad // 8, T, 100, 8], BF16,
                          kind="ExternalInput").ap()
    xr_d = nc.dram_tensor("xr", [CC, 128, T, npad // 8, 512], F32,
                          kind="ExternalInput").ap()
    wq_d = nc.dram_tensor("wq", [CC, 128, 9, C], BF16, kind="ExternalInput").ap()
    wk_d = nc.dram_tensor("wk", [CC, 128, 9, C], BF16, kind="ExternalInput").ap()
    wv_d = nc.dram_tensor("wv", [CC, 128, 9, C], BF16, kind="ExternalInput").ap()
    wc_d = nc.dram_tensor("wc", [CC, 128, 9, C], BF16, kind="ExternalInput").ap()
    mask_d = nc.dram_tensor("mask", [NA], F32, kind="ExternalInput").ap()
    gamma_d = nc.dram_tensor("gamma", [C], F32, kind="ExternalInput").ap()
    beta_d = nc.dram_tensor("beta", [C], F32, kind="ExternalInput").ap()
    out_d = nc.dram_tensor("out", [CC, 128, T, npad // 8, 512], F32,
                           kind="ExternalOutput").ap()

    def bcast_ap(src, n_part, extra):
        return bass.AP(tensor=src.tensor, offset=src.offset,
                       ap=[[0, n_part]] + extra)

    with tile.TileContext(nc) as tc:
        with (
            tc.tile_pool(name="singles", bufs=1) as singles,
            tc.tile_pool(name="dram", bufs=1, space="DRAM") as dpool,
        ):
            ident = singles.tile([128, 128], F32)
            make_identity(nc, ident)
            ident_bf = singles.tile([128, 128], BF16)
            nc.vector.tensor_copy(out=ident_bf, in_=ident)
            mask_b = singles.tile([128, NA], F32)
            nc.gpsimd.dma_start(out=mask_b,
                                in_=bcast_ap(mask_d, 128, [[1, NA]]))
            zeros1 = singles.tile([128, 1], F32)
            nc.vector.memset(zeros1, 0.0)
            eps_t = singles.tile([64, 1], F32)
            nc.vector.memset(eps_t, GN_EPS)
            gam = singles.tile([128, CC], F32)
            bet = singles.tile([128, CC], F32)
            for cc in range(CC):
                nc.sync.dma_start(out=gam[:, cc:cc + 1],
                                  in_=gamma_d[cc * 128:(cc + 1) * 128])
                nc.sync.dma_start(out=bet[:, cc:cc + 1],
                                  in_=beta_d[cc * 128:(cc + 1) * 128])
            # per-(i, pair) bn stats: partitions 0:64 = even locs, 64:128 = odd
            stats = singles.tile([128, 128, 6], F32)

            vsp = dpool.tile([T, 32, 128, C], F32)
            mvd = dpool.tile([128, 2], F32)
            gnd = dpool.tile([64, 2], F32)

            # ============ Fused conv(q,k,v) + attention, per t ============
            with (
                tc.tile_pool(name="wA", bufs=1) as wpool,
                tc.tile_pool(name="xA", bufs=1) as xpool,
                tc.tile_pool(name="qkvB", bufs=1) as qkv,
                tc.tile_pool(name="sbB", bufs=2) as pB,
                tc.tile_pool(name="psAB", bufs=1, space="PSUM") as psum,
            ):
                for t in range(T):
                    # x tiles for this t (shared by all three convs)
                    xs = {}
                    for nb in range(nbk):
                        for cc in range(CC):
                            xt = xpool.tile([128, 100, 8], BF16,
                                            tag=f"x{nb}_{cc}",
                                            name=f"x{nb}_{cc}")
                            nc.sync.dma_start(out=xt, in_=xp_d[cc, :, nb, t])
                            xs[(nb, cc)] = xt
                    # qkv half tiles [c, 32hw, 64n], zero pad rows
                    qkv_sb = {}
                    for name, wd in (("q", wq_d), ("k", wk_d), ("v", wv_d)):
                        w_sb = []
                        for cc in range(CC):
                            wt = wpool.tile([128, 9, C], BF16,
                                            tag=f"w{cc}", name=f"w{cc}")
                            nc.sync.dma_start(out=wt, in_=wd[cc])
                            w_sb.append(wt)
                        for half in range(2):
                            for cc in range(CC):
                                tl = qkv.tile([128, 32, NA], BF16,
                                              tag=f"{name}{half}{cc}",
                                              name=f"{name}{half}{cc}")
                                if npad < NA:
                                    nc.vector.tensor_copy(
                                        out=tl[:, :, npad:NA],
                                        in_=bass.AP(
                                            tensor=zeros1.tensor,
                                            offset=zeros1.offset,
                                            ap=[zeros1.ap[0], [0, 32],
                                                [0, NA - npad]]),
                                    )
                                qkv_sb[(name, half, cc)] = tl
                        for nb in range(nbk):
                            for oc in range(4):
                                ps = psum.tile([128, 512], F32, tag="cps",
                                               bufs=2)
                                for cc in range(CC):
                                    for s in range(9):
                                        dh, dw = s // 3, s % 3
                                        xt = xs[(nb, cc)]
                                        rhs = bass.AP(
                                            tensor=xt.tensor,
                                            offset=xt.offset
                                            + (dh * 10 + dw) * 8,
                                            ap=[xt.ap[0], [80, 8], [8, 8],
                                                [1, 8]],
                                        )
                                        nc.tensor.matmul(
                                            ps,
                                            lhsT=w_sb[cc][:, s,
                                                          oc * 128:(oc + 1) * 128],
                                            rhs=rhs,
                                            start=(cc == 0 and s == 0),
                                            stop=(cc == CC - 1 and s == 8),
                                        )
                                # copy PSUM (h,w,n) into the half tiles
                                for half in range(2):
                                    tl = qkv_sb[(name, half, oc)]
                                    src_ap = bass.AP(
                                        tensor=ps.tensor,
                                        offset=ps.offset + half * 32 * 8,
                                        ap=[ps.ap[0], [8, 32], [1, 8]])
                                    dst_ap = bass.AP(
                                        tensor=tl.tensor,
                                        offset=tl.offset + nb * 8,
                                        ap=[tl.ap[0], [NA, 32], [1, 8]])
                                    nc.vector.tensor_copy(out=dst_ap,
                                                          in_=src_ap)
                    # ---- attention for this t ----
                    for half in range(2):
                        q_sb = [qkv_sb[("q", half, cc)] for cc in range(CC)]
                        k_sb = [qkv_sb[("k", half, cc)] for cc in range(CC)]
                        v_sb = [qkv_sb[("v", half, cc)] for cc in range(CC)]
                        for quad in range(8):
                            h4 = quad * 4
                            s_ab = []
                            for sub in range(2):
                                sp = psum.tile([128, 256], F32, tag=f"s{sub}",
                                               bufs=1)
                                for cc in range(CC):
                                    nc.tensor.matmul(
                                        sp,
                                        lhsT=q_sb[cc][:, h4 + 2 * sub:
                                                      h4 + 2 * sub + 2, :],
                                        rhs=k_sb[cc][:, h4:h4 + 4, :],
                                        start=(cc == 0), stop=(cc == CC - 1),
                                    )
                                s_ab.append(sp)
                            for pp in range(2):
                                s_ps = s_ab[pp]
                                pair = t * 32 + half * 16 + quad * 2 + pp
                                e_sb = pB.tile([128, 128], BF16, tag="e")
                                nc.vector.memset(e_sb[0:64, 64:128], 0.0)
                                nc.vector.memset(e_sb[64:128, 0:64], 0.0)
                                nm = pB.tile([128, 1], F32, tag="nm")
                                dsum = pB.tile([128, 1], F32, tag="d")
                                rr = pB.tile([128, 1], F32, tag="r")
                                for l in range(2):
                                    rs = slice(64 * l, 64 * l + 64)
                                    cs = slice(128 * pp + 64 * l,
                                               128 * pp + 64 * l + 64)
                                    sm = pB.tile([128, 64], F32, tag="sm")
                                    nc.vector.tensor_tensor(
                                        out=sm[rs], in0=s_ps[rs, cs],
                                        in1=mask_b[rs],
                                        op=mybir.AluOpType.add)
                                    nc.vector.tensor_reduce(
                                        out=nm[rs], in_=sm[rs],
                                        axis=mybir.AxisListType.X,
                                        op=mybir.AluOpType.max, negate=True)
                                    nc.scalar.activation(
                                        out=e_sb[rs, rs], in_=sm[rs],
                                        func=mybir.ActivationFunctionType.Exp,
                                        bias=nm[rs], scale=1.0,
                                        accum_out=dsum[rs])
                                nc.vector.reciprocal(out=rr, in_=dsum)

                                et_ps = psum.tile([128, 128], BF16,
                                                  tag="et_ps", bufs=1)
                                nc.tensor.transpose(et_ps, e_sb, ident_bf)
                                et = pB.tile([128, 128], BF16, tag="et")
                                nc.vector.tensor_copy(out=et, in_=et_ps)

                                vpair = pB.tile([128, C], BF16, tag="vp")
                                for cc in range(CC):
                                    vt_ps = psum.tile([128, 128], BF16,
                                                      tag="vt_ps", bufs=2)
                                    nc.tensor.transpose(
                                        vt_ps,
                                        v_sb[cc][:, h4 + 2 * pp:
                                                 h4 + 2 * pp + 2, :],
                                        ident_bf)
                                    nc.vector.tensor_copy(
                                        out=vpair[:, cc * 128:(cc + 1) * 128],
                                        in_=vt_ps)

                                av_ps = psum.tile([128, C], F32, tag="av",
                                                  bufs=1)
                                nc.tensor.matmul(av_ps, lhsT=et, rhs=vpair,
                                                 start=True, stop=True)
                                vb = pB.tile([128, C], F32, tag="vb")
                                nc.vector.tensor_scalar_mul(vb, av_ps, rr)
                                nc.vector.bn_stats(out=stats[:, pair, :],
                                                   in_=vb)
                                nc.sync.dma_start(
                                    out=vsp[t, half * 16 + quad * 2 + pp],
                                    in_=vb)

                # ---- GroupNorm stats finalize ----
                mv = pB.tile([128, 2], F32, tag="mv")
                nc.vector.bn_aggr(out=mv, in_=stats)
                nc.sync.dma_start(out=mvd, in_=mv)
                mva = pB.tile([64, 2], F32, tag="mva")
                mvb = pB.tile([64, 2], F32, tag="mvb")
                nc.sync.dma_start(out=mva, in_=mvd[0:64])
                nc.sync.dma_start(out=mvb, in_=mvd[64:128])
                mu = pB.tile([64, 1], F32, tag="mu")
                nc.vector.tensor_add(mu, mva[:, 0:1], mvb[:, 0:1])
                nc.vector.tensor_scalar_mul(mu, mu, 0.5)
                dm = pB.tile([64, 1], F32, tag="dm")
                nc.vector.tensor_sub(dm, mva[:, 0:1], mvb[:, 0:1])
                nc.vector.tensor_scalar_mul(dm, dm, 0.5)
                nc.vector.tensor_mul(dm, dm, dm)
                var = pB.tile([64, 1], F32, tag="var")
                nc.vector.tensor_add(var, mva[:, 1:2], mvb[:, 1:2])
                nc.vector.tensor_scalar_mul(var, var, 0.5)
                nc.vector.tensor_add(var, var, dm)
                rstd = pB.tile([64, 1], F32, tag="rstd")
                nc.scalar.activation(out=rstd, in_=var,
                                     func=mybir.ActivationFunctionType.Sqrt,
                                     bias=eps_t, scale=1.0)
                nc.vector.reciprocal(out=rstd, in_=rstd)
                murstd = pB.tile([64, 1], F32, tag="murstd")
                nc.vector.tensor_mul(murstd, mu, rstd)
                gpack = pB.tile([64, 2], F32, tag="gpack")
                nc.vector.tensor_copy(out=gpack[:, 0:1], in_=rstd)
                nc.vector.tensor_copy(out=gpack[:, 1:2], in_=murstd)
                nc.sync.dma_start(out=gnd, in_=gpack)

            # broadcast (rstd, mu*rstd) along partitions: [128, 64, 2]
            abn = singles.tile([128, 64, 2], F32)
            nc.gpsimd.dma_start(out=abn,
                                in_=bcast_ap(gnd, 128, [[2, 64], [1, 2]]))

            # ================= Phase C: GN apply + Wc conv + residual ====
            with (
                tc.tile_pool(name="wC", bufs=1) as wpool,
                tc.tile_pool(name="vtC", bufs=3) as vtpool,
                tc.tile_pool(name="padC", bufs=1) as padpool,
                tc.tile_pool(name="ioC", bufs=3) as iopool,
                tc.tile_pool(name="psC", bufs=1, space="PSUM") as psC,
            ):
                wc_sb = []
                for cc in range(CC):
                    wt = wpool.tile([128, 9, C], BF16, tag=f"wc{cc}")
                    nc.sync.dma_start(out=wt, in_=wc_d[cc])
                    wc_sb.append(wt)
                # persistent pre-zeroed padded tiles (double-buffered manually)
                vpads = []
                for par in range(2):
                    row = []
                    for cc in range(CC):
                        vp = padpool.tile([128, 100, 8], BF16,
                                          tag=f"vp{par}_{cc}")
                        nc.vector.tensor_copy(
                            out=vp,
                            in_=bass.AP(tensor=zeros1.tensor,
                                        offset=zeros1.offset,
                                        ap=[zeros1.ap[0], [0, 100], [0, 8]]))
                        row.append(vp)
                    vpads.append(row)
                it_c = 0
                for t in range(T):
                    vt_big = [vtpool.tile([128, 32, 2, NA], BF16,
                                          tag=f"vt{cc}", name=f"vt{cc}")
                              for cc in range(CC)]
                    for pr in range(32):
                        vb_r = iopool.tile([128, C], F32, tag="vbr",
                                           bufs=6)
                        nc.scalar.dma_start(out=vb_r, in_=vsp[t, pr])
                        for cc in range(CC):
                            tp = psC.tile([128, 128], F32, tag="tp", bufs=2)
                            nc.tensor.transpose(
                                tp, vb_r[:, cc * 128:(cc + 1) * 128], ident)
                            nc.vector.tensor_copy(out=vt_big[cc][:, pr, :, :],
                                                  in_=tp)
                    for cc in range(CC):
                        vt = vt_big[cc]
                        rstd_b = bass.AP(tensor=abn.tensor, offset=abn.offset,
                                         ap=[abn.ap[0], [0, 32], [0, 2],
                                             [2, NA]])
                        murstd_b = bass.AP(tensor=abn.tensor,
                                           offset=abn.offset + 1,
                                           ap=[abn.ap[0], [0, 32], [0, 2],
                                               [2, NA]])
                        nc.vector.tensor_tensor(out=vt, in0=vt, in1=rstd_b,
                                                op=mybir.AluOpType.mult)
                        nc.vector.tensor_tensor(out=vt, in0=vt, in1=murstd_b,
                                                op=mybir.AluOpType.subtract)
                        nc.vector.tensor_scalar(
                            out=vt, in0=vt, scalar1=gam[:, cc:cc + 1],
                            scalar2=bet[:, cc:cc + 1],
                            op0=mybir.AluOpType.mult,
                            op1=mybir.AluOpType.add)
                        nc.scalar.activation(
                            out=vt, in_=vt,
                            func=mybir.ActivationFunctionType.Relu)
                    for nb in range(nbk):
                        par = it_c % 2
                        it_c += 1
                        for cc in range(CC):
                            src = bass.AP(
                                tensor=vt_big[cc].tensor,
                                offset=vt_big[cc].offset + nb * 8,
                                ap=[vt_big[cc].ap[0], [512, 8], [64, 8],
                                    [1, 8]])
                            dst = bass.AP(
                                tensor=vpads[par][cc].tensor,
                                offset=vpads[par][cc].offset + 11 * 8,
                                ap=[vpads[par][cc].ap[0], [80, 8], [8, 8],
                                    [1, 8]])
                            nc.vector.tensor_copy(out=dst, in_=src)
                        for oc in range(4):
                            ps = psC.tile([128, 512], F32, tag="cps", bufs=5)
                            for cc in range(CC):
                                for s in range(9):
                                    dh, dw = s // 3, s % 3
                                    vp = vpads[par][cc]
                                    rhs = bass.AP(
                                        tensor=vp.tensor,
                                        offset=vp.offset + (dh * 10 + dw) * 8,
                                        ap=[vp.ap[0], [80, 8], [8, 8],
                                            [1, 8]])
                                    nc.tensor.matmul(
                                        ps,
                                        lhsT=wc_sb[cc][:, s,
                                                       oc * 128:(oc + 1) * 128],
                                        rhs=rhs,
                                        start=(cc == 0 and s == 0),
                                        stop=(cc == CC - 1 and s == 8))
                            xr = iopool.tile([128, 512], F32, tag="xr")
                            nc.sync.dma_start(out=xr,
                                              in_=xr_d[oc, :, t, nb])
                            ob = iopool.tile([128, 512], F32, tag="cob")
                            nc.vector.tensor_add(ob, ps, xr)
                            nc.sync.dma_start(out=out_d[oc, :, t, nb],
                                               in_=ob)

    nc.compile()
    return nc


_BUILD_CACHE = {}


def kernel(x, roi_inds, Wq, Wk, Wv, Wc, gn_gamma, gn_beta):
    global LAST_EXEC_NS
    x = np.ascontiguousarray(np.asarray(x, dtype=np.float32))
    roi_inds = np.asarray(roi_inds, dtype=np.int32)
    n, c = x.shape[0], x.shape[1]
    assert (n, c) == (N, C) and x.shape[2:] == (T, H, W)

    # group ROIs per clip; core g <- group g
    order = np.argsort(roi_inds, kind="stable")
    groups = [order[roi_inds[order] == g] for g in range(N_CORES)]
    sizes = [len(g) for g in groups]
    max_sz = max(sizes)
    assert max_sz <= NA, f"group size {max_sz} > {NA} unsupported"
    npad = ((max_sz + 7) // 8) * 8

    scale = 1.0 / np.sqrt(np.float32(C))

    def prep_w(Wt, sc=1.0):
        # [O, C, 1, 3, 3] -> [CC, 128, 9, O]
        w = (np.asarray(Wt, np.float32)[:, :, 0] * sc)  # [O, C, 3, 3]
        w = w.transpose(1, 2, 3, 0).reshape(CC, 128, 9, C)
        return np.ascontiguousarray(w).astype(ml_dtypes.bfloat16)

    w_arrs = {
        "wq": prep_w(Wq, scale), "wk": prep_w(Wk), "wv": prep_w(Wv),
        "wc": prep_w(Wc),
    }
    gamma = np.ascontiguousarray(np.asarray(gn_gamma, np.float32))
    beta = np.ascontiguousarray(np.asarray(gn_beta, np.float32))

    in_maps = []
    for g in range(N_CORES):
        idx = groups[g]
        xg = np.zeros((npad, C, T, H, W), np.float32)
        xg[:sizes[g]] = x[idx]
        # xp: [CC, 128, npad, T, 10, 10] zero-padded -> [CC,128,npad,T,100]
        xcm = xg.transpose(1, 0, 2, 3, 4).reshape(CC, 128, npad, T, H, W)
        xpp = np.zeros((CC, 128, npad, T, 10, 10), np.float32)
        xpp[..., 1:9, 1:9] = xcm
        # -> [CC, 128, NBK, T, 100, 8] with n innermost (contiguous for PE)
        xpp = xpp.reshape(CC, 128, npad // 8, 8, T, 100)
        xp = np.ascontiguousarray(
            xpp.transpose(0, 1, 2, 4, 5, 3)).astype(ml_dtypes.bfloat16)
        # xr: [CC, 128, T, NBK, (h,w,n8)] matching conv PSUM column order
        xr = np.ascontiguousarray(
            xcm.reshape(CC, 128, npad // 8, 8, T, HW)
            .transpose(0, 1, 4, 2, 5, 3)).reshape(
                CC, 128, T, npad // 8, 512)
        mask = np.zeros(NA, np.float32)
        mask[sizes[g]:] = -1e30
        m = {"xp": xp, "xr": xr, "mask": mask,
             "gamma": gamma, "beta": beta}
        m.update(w_arrs)
        in_maps.append(m)

    if npad not in _BUILD_CACHE:
        _BUILD_CACHE[npad] = _build(npad)
    nc = _BUILD_CACHE[npad]

    trace = bool(int(os.environ.get("ACAR_TRACE", "0")))
    if trace:
        _install_ntff_hook()
    res = run_bass_kernel_spmd(nc, in_maps, core_ids=list(range(N_CORES)),
                               trace=trace)
    LAST_EXEC_NS = res.exec_time_ns

    out = np.empty((N, C, T, H, W), np.float32)
    for g in range(N_CORES):
        og = res.results[g]["out"]  # [CC, 128, T, NBK, (hw, n8)]
        og = og.reshape(C, T, npad // 8, HW, 8)
        og = og.transpose(2, 4, 0, 1, 3).reshape(npad, C, T, H, W)
        out[groups[g]] = og[:sizes[g]]
    return out


# revision 12
# speedup vs baseline: 1.2060x; 1.0085x over previous
"""ACAR head (grouped ROI attention) Trainium2 Bass kernel.

Strategy: data-parallel over ROI groups. roi_inds has NUM_CLIPS=8 groups and
there are 8 NeuronCores, so core c owns group c (padded to a common Npad).
Attention never crosses groups, so there is no inter-core communication; the
host shards inputs / gathers outputs.

Compute dtype: float32r (TF32-like rounded fp32) — full-rate on the PE at
free-dim >= 256, ~1.5e-4 relative rounding.
"""

import os
import sys
import types

sys.path.insert(0, "/opt/trn_rl_repo")

import numpy as np
import ml_dtypes


def _install_ntff_hook():
    """The image's antenv package lacks axon_hooks; inject it so trace=True
    can capture NTFF profiles. Harmless if anything is missing."""
    try:
        import antenv  # noqa: F401
        from trn_agent_boot.trn_boot import _ntff_profile_via_ctypes

        hook = _ntff_profile_via_ctypes("/opt/axon/libaxon_pjrt.so")
        if hook is None:
            return False
        mod = types.ModuleType("antenv.axon_hooks")
        mod.get_axon_ntff_profile_hook = lambda: hook
        mod.set_axon_ntff_profile_hook = lambda h: None
        sys.modules["antenv.axon_hooks"] = mod
        return True
    except Exception:
        return False


import concourse.bass as bass
import concourse.bacc as bacc
import concourse.tile as tile
from concourse import mybir
from concourse.bass_utils import run_bass_kernel_spmd
from concourse.masks import make_identity

F32 = mybir.dt.float32
F32R = mybir.dt.float32r
BF16 = mybir.dt.bfloat16

N_CORES = 8
N, C, T, H, W = 256, 512, 4, 8, 8
HW = H * W
CC = C // 128          # c chunks
NA = 64                # attention row pad (group size must be <= 64)
GN_EPS = 1e-5

LAST_EXEC_NS = None


def _build(npad: int):
    nbk = npad // 8
    nc = bacc.Bacc("TRN2", target_bir_lowering=False, debug=False,
                   num_devices=N_CORES)

    # ---- dram parameters (per-core shards) ----
    xp_d = nc.dram_tensor("xp", [CC, 128, npad // 8, T, 100, 8], BF16,
                          kind="ExternalInput").ap()
    xr_d = nc.dram_tensor("xr", [CC, 128, T, npad // 8, 512], F32,
                          kind="ExternalInput").ap()
    wq_d = nc.dram_tensor("wq", [CC, 128, 9, C], BF16, kind="ExternalInput").ap()
    wk_d = nc.dram_tensor("wk", [CC, 128, 9, C], BF16, kind="ExternalInput").ap()
    wv_d = nc.dram_tensor("wv", [CC, 128, 9, C], BF16, kind="ExternalInput").ap()
    wc_d = nc.dram_tensor("wc", [CC, 128, 9, C], BF16, kind="ExternalInput").ap()
    mask_d = nc.dram_tensor("mask", [NA], F32, kind="ExternalInput").ap()
    gamma_d = nc.dram_tensor("gamma", [C], F32, kind="ExternalInput").ap()
    beta_d = nc.dram_tensor("beta", [C], F32, kind="ExternalInput").ap()
    out_d = nc.dram_tensor("out", [CC, 128, T, npad // 8, 512], F32,
                           kind="ExternalOutput").ap()

    def bcast_ap(src, n_part, extra):
        return bass.AP(tensor=src.tensor, offset=src.offset,
                       ap=[[0, n_part]] + extra)

    with tile.TileContext(nc) as tc:
        with (
            tc.tile_pool(name="singles", bufs=1) as singles,
            tc.tile_pool(name="dram", bufs=1, space="DRAM") as dpool,
        ):
            ident = singles.tile([128, 128], F32)
            make_identity(nc, ident)
            ident_bf = singles.tile([128, 128], BF16)
            nc.vector.tensor_copy(out=ident_bf, in_=ident)
            mask_b = singles.tile([128, NA], F32)
            nc.gpsimd.dma_start(out=mask_b,
                                in_=bcast_ap(mask_d, 128, [[1, NA]]))
            zeros1 = singles.tile([128, 1], F32)
            nc.vector.memset(zeros1, 0.0)
            eps_t = singles.tile([64, 1], F32)
            nc.vector.memset(eps_t, GN_EPS)
            gam = singles.tile([128, CC], F32)
            bet = singles.tile([128, CC], F32)
            for cc in range(CC):
                nc.sync.dma_start(out=gam[:, cc:cc + 1],
                                  in_=gamma_d[cc * 128:(cc + 1) * 128])
                nc.sync.dma_start(out=bet[:, cc:cc + 1],
                                  in_=beta_d[cc * 128:(cc + 1) * 128])
            # per-(i, pair) bn stats: partitions 0:64 = even locs, 64:128 = odd
            stats = singles.tile([128, 128, 6], F32)

            vsp = dpool.tile([T, 32, 128, C], F32)
            mvd = dpool.tile([128, 2], F32)
            gnd = dpool.tile([64, 2], F32)

            # ============ Fused conv(q,k,v) + attention, per t ============
            with (
                tc.tile_pool(name="wA", bufs=1) as wpool,
                tc.tile_pool(name="xA", bufs=1) as xpool,
                tc.tile_pool(name="qkvB", bufs=1) as qkv,
                tc.tile_pool(name="sbB", bufs=2) as pB,
                tc.tile_pool(name="psAB", bufs=1, space="PSUM") as psum,
            ):
                for t in range(T):
                    # x tiles for this t (shared by all three convs)
                    xs = {}
                    for nb in range(nbk):
                        for cc in range(CC):
                            xt = xpool.tile([128, 100, 8], BF16,
                                            tag=f"x{nb}_{cc}",
                                            name=f"x{nb}_{cc}")
                            nc.sync.dma_start(out=xt, in_=xp_d[cc, :, nb, t])
                            xs[(nb, cc)] = xt
                    # qkv half tiles [c, 32hw, 64n], zero pad rows
                    qkv_sb = {}
                    for name, wd in (("q", wq_d), ("k", wk_d), ("v", wv_d)):
                        w_sb = []
                        for cc in range(CC):
                            wt = wpool.tile([128, 9, C], BF16,
                                            tag=f"w{cc}", name=f"w{cc}")
                            nc.sync.dma_start(out=wt, in_=wd[cc])
                            w_sb.append(wt)
                        for half in range(2):
                            for cc in range(CC):
                                tl = qkv.tile([128, 32, NA], BF16,
                                              tag=f"{name}{half}{cc}",
                                              name=f"{name}{half}{cc}")
                                if npad < NA:
                                    nc.vector.tensor_copy(
                                        out=tl[:, :, npad:NA],
                                        in_=bass.AP(
                                            tensor=zeros1.tensor,
                                            offset=zeros1.offset,
                                            ap=[zeros1.ap[0], [0, 32],
                                                [0, NA - npad]]),
                                    )
                                qkv_sb[(name, half, cc)] = tl
                        for nb in range(nbk):
                            for oc in range(4):
                                ps = psum.tile([128, 512], F32, tag="cps",
                                               bufs=2)
                                for cc in range(CC):
                                    for s in range(9):
                                        dh, dw = s // 3, s % 3
                                        xt = xs[(nb, cc)]
                                        rhs = bass.AP(
                                            tensor=xt.tensor,
                                            offset=xt.offset
                                            + (dh * 10 + dw) * 8,
                                            ap=[xt.ap[0], [80, 8], [8, 8],
                                                [1, 8]],
                                        )
                                        nc.tensor.matmul(
                                            ps,
                                            lhsT=w_sb[cc][:, s,
                                                          oc * 128:(oc + 1) * 128],
                                            rhs=rhs,
                                            start=(cc == 0 and s == 0),
                                            stop=(cc == CC - 1 and s == 8),
                                        )
                                # copy PSUM (h,w,n) into the half tiles
                                for half in range(2):
                                    tl = qkv_sb[(name, half, oc)]
                                    src_ap = bass.AP(
                                        tensor=ps.tensor,
                                        offset=ps.offset + half * 32 * 8,
                                        ap=[ps.ap[0], [8, 32], [1, 8]])
                                    dst_ap = bass.AP(
                                        tensor=tl.tensor,
                                        offset=tl.offset + nb * 8,
                                        ap=[tl.ap[0], [NA, 32], [1, 8]])
                                    nc.vector.tensor_copy(out=dst_ap,
                                                          in_=src_ap)
                    # ---- attention for this t ----
                    for half in range(2):
                        q_sb = [qkv_sb[("q", half, cc)] for cc in range(CC)]
                        k_sb = [qkv_sb[("k", half, cc)] for cc in range(CC)]
                        v_sb = [qkv_sb[("v", half, cc)] for cc in range(CC)]
                        for quad in range(8):
                            h4 = quad * 4
                            s_ab = []
                            for sub in range(2):
                                sp = psum.tile([128, 256], F32, tag=f"s{sub}",
                                               bufs=1)
                                for cc in range(CC):
                                    nc.tensor.matmul(
                                        sp,
                                        lhsT=q_sb[cc][:, h4 + 2 * sub:
                                                      h4 + 2 * sub + 2, :],
                                        rhs=k_sb[cc][:, h4:h4 + 4, :],
                                        start=(cc == 0), stop=(cc == CC - 1),
                                    )
                                s_ab.append(sp)
                            for pp in range(2):
                                s_ps = s_ab[pp]
                                pair = t * 32 + half * 16 + quad * 2 + pp
                                e_sb = pB.tile([128, 128], BF16, tag="e")
                                nc.vector.memset(e_sb[0:64, 64:128], 0.0)
                                nc.vector.memset(e_sb[64:128, 0:64], 0.0)
                                nm = pB.tile([128, 1], F32, tag="nm")
                                dsum = pB.tile([128, 1], F32, tag="d")
                                rr = pB.tile([128, 1], F32, tag="r")
                                for l in range(2):
                                    rs = slice(64 * l, 64 * l + 64)
                                    cs = slice(128 * pp + 64 * l,
                                               128 * pp + 64 * l + 64)
                                    sm = pB.tile([128, 64], F32, tag="sm")
                                    nc.vector.tensor_tensor(
                                        out=sm[rs], in0=s_ps[rs, cs],
                                        in1=mask_b[rs],
                                        op=mybir.AluOpType.add)
                                    nc.vector.tensor_reduce(
                                        out=nm[rs], in_=sm[rs],
                                        axis=mybir.AxisListType.X,
                                        op=mybir.AluOpType.max, negate=True)
                                    nc.scalar.activation(
                                        out=e_sb[rs, rs], in_=sm[rs],
                                        func=mybir.ActivationFunctionType.Exp,
                                        bias=nm[rs], scale=1.0,
                                        accum_out=dsum[rs])
                                nc.vector.reciprocal(out=rr, in_=dsum)

                                et_ps = psum.tile([128, 128], BF16,
                                                  tag="et_ps", bufs=1)
                                nc.tensor.transpose(et_ps, e_sb, ident_bf)
                                et = pB.tile([128, 128], BF16, tag="et")
                                nc.vector.tensor_copy(out=et, in_=et_ps)

                                vpair = pB.tile([128, C], BF16, tag="vp")
                                for cc in range(CC):
                                    vt_ps = psum.tile([128, 128], BF16,
                                                      tag="vt_ps", bufs=2)
                                    nc.tensor.transpose(
                                        vt_ps,
                                        v_sb[cc][:, h4 + 2 * pp:
                                                 h4 + 2 * pp + 2, :],
                                        ident_bf)
                                    nc.vector.tensor_copy(
                                        out=vpair[:, cc * 128:(cc + 1) * 128],
                                        in_=vt_ps)

                                av_ps = psum.tile([128, C], F32, tag="av",
                                                  bufs=1)
                                nc.tensor.matmul(av_ps, lhsT=et, rhs=vpair,
                                                 start=True, stop=True)
                                vb = pB.tile([128, C], F32, tag="vb")
                                nc.vector.tensor_scalar_mul(vb, av_ps, rr)
                                nc.vector.bn_stats(out=stats[:, pair, :],
                                                   in_=vb)
                                nc.sync.dma_start(
                                    out=vsp[t, half * 16 + quad * 2 + pp],
                                    in_=vb)

                # ---- GroupNorm stats finalize ----
                mv = pB.tile([128, 2], F32, tag="mv")
                nc.vector.bn_aggr(out=mv, in_=stats)
                nc.sync.dma_start(out=mvd, in_=mv)
                mva = pB.tile([64, 2], F32, tag="mva")
                mvb = pB.tile([64, 2], F32, tag="mvb")
                nc.sync.dma_start(out=mva, in_=mvd[0:64])
                nc.sync.dma_start(out=mvb, in_=mvd[64:128])
                mu = pB.tile([64, 1], F32, tag="mu")
                nc.vector.tensor_add(mu, mva[:, 0:1], mvb[:, 0:1])
                nc.vector.tensor_scalar_mul(mu, mu, 0.5)
                dm = pB.tile([64, 1], F32, tag="dm")
                nc.vector.tensor_sub(dm, mva[:, 0:1], mvb[:, 0:1])
                nc.vector.tensor_scalar_mul(dm, dm, 0.5)
                nc.vector.tensor_mul(dm, dm, dm)
                var = pB.tile([64, 1], F32, tag="var")
                nc.vector.tensor_add(var, mva[:, 1:2], mvb[:, 1:2])
                nc.vector.tensor_scalar_mul(var, var, 0.5)
                nc.vector.tensor_add(var, var, dm)
                rstd = pB.tile([64, 1], F32, tag="rstd")
                nc.scalar.activation(out=rstd, in_=var,
                                     func=mybir.ActivationFunctionType.Sqrt,
                                     bias=eps_t, scale=1.0)
                nc.vector.reciprocal(out=rstd, in_=rstd)
                murstd = pB.tile([64, 1], F32, tag="murstd")
                nc.vector.tensor_mul(murstd, mu, rstd)
                gpack = pB.tile([64, 2], F32, tag="gpack")
                nc.vector.tensor_copy(out=gpack[:, 0:1], in_=rstd)
                nc.vector.tensor_copy(out=gpack[:, 1:2], in_=murstd)
                nc.sync.dma_start(out=gnd, in_=gpack)

            # broadcast (rstd, mu*rstd) along partitions: [128, 64, 2]
            abn = singles.tile([128, 64, 2], F32)
            nc.gpsimd.dma_start(out=abn,
                                in_=bcast_ap(gnd, 128, [[2, 64], [1, 2]]))

            # ================= Phase C: GN apply + Wc conv + residual ====
            with (
                tc.tile_pool(name="wC", bufs=1) as wpool,
                tc.tile_pool(name="vtC", bufs=2) as vtpool,
                tc.tile_pool(name="padC", bufs=1) as padpool,
                tc.tile_pool(name="ioC", bufs=3) as iopool,
                tc.tile_pool(name="psC", bufs=1, space="PSUM") as psC,
            ):
                wc_sb = []
                for cc in range(CC):
                    wt = wpool.tile([128, 9, C], BF16, tag=f"wc{cc}")
                    nc.sync.dma_start(out=wt, in_=wc_d[cc])
                    wc_sb.append(wt)
                # persistent pre-zeroed padded tiles (double-buffered manually)
                vpads = []
                for par in range(2):
                    row = []
                    for cc in range(CC):
                        vp = padpool.tile([128, 100, 8], BF16,
                                          tag=f"vp{par}_{cc}")
                        nc.vector.tensor_copy(
                            out=vp,
                            in_=bass.AP(tensor=zeros1.tensor,
                                        offset=zeros1.offset,
                                        ap=[zeros1.ap[0], [0, 100], [0, 8]]))
                        row.append(vp)
                    vpads.append(row)
                it_c = 0
                for t in range(T):
                    vt_big = [vtpool.tile([128, 32, 2, NA], BF16,
                                          tag=f"vt{cc}", name=f"vt{cc}")
                              for cc in range(CC)]
                    for pr in range(32):
                        vb_r = iopool.tile([128, C], F32, tag="vbr",
                                           bufs=6)
                        nc.scalar.dma_start(out=vb_r, in_=vsp[t, pr])
                        for cc in range(CC):
                            tp = psC.tile([128, 128], F32, tag="tp", bufs=2)
                            nc.tensor.transpose(
                                tp, vb_r[:, cc * 128:(cc + 1) * 128], ident)
                            nc.vector.tensor_copy(out=vt_big[cc][:, pr, :, :],
                                                  in_=tp)
                    for cc in range(CC):
                        vt = vt_big[cc]
                        rstd_b = bass.AP(tensor=abn.tensor, offset=abn.offset,
                                         ap=[abn.ap[0], [0, 32], [0, 2],
                                             [2, NA]])
                        murstd_b = bass.AP(tensor=abn.tensor,
                                           offset=abn.offset + 1,
                                           ap=[abn.ap[0], [0, 32], [0, 2],
                                               [2, NA]])
                        nc.vector.tensor_tensor(out=vt, in0=vt, in1=rstd_b,
                                                op=mybir.AluOpType.mult)
                        nc.vector.tensor_tensor(out=vt, in0=vt, in1=murstd_b,
                                                op=mybir.AluOpType.subtract)
                        nc.vector.tensor_scalar(
                            out=vt, in0=vt, scalar1=gam[:, cc:cc + 1],
                            scalar2=bet[:, cc:cc + 1],
                            op0=mybir.AluOpType.mult,
                            op1=mybir.AluOpType.add)
                        nc.scalar.activation(
                            out=vt, in_=vt,
                            func=mybir.ActivationFunctionType.Relu)
                    for nb in range(nbk):
                        par = it_c % 2
                        it_c += 1
                        for cc in range(CC):
                            src = bass.AP(
                                tensor=vt_big[cc].tensor,
                                offset=vt_big[cc].offset + nb * 8,
                                ap=[vt_big[cc].ap[0], [512, 8], [64, 8],
                                    [1, 8]])
                            dst = bass.AP(
                                tensor=vpads[par][cc].tensor,
                                offset=vpads[par][cc].offset + 11 * 8,
                                ap=[vpads[par][cc].ap[0], [80, 8], [8, 8],
                                    [1, 8]])
                            nc.vector.tensor_copy(out=dst, in_=src)
                        for oc in range(4):
                            ps = psC.tile([128, 512], F32, tag="cps", bufs=4)
                            for cc in range(CC):
                                for s in range(9):
                                    dh, dw = s // 3, s % 3
                                    vp = vpads[par][cc]
                                    rhs = bass.AP(
                                        tensor=vp.tensor,
                                        offset=vp.offset + (dh * 10 + dw) * 8,
                                        ap=[vp.ap[0], [80, 8], [8, 8],
                                            [1, 8]])
                                    nc.tensor.matmul(
                                        ps,
                                        lhsT=wc_sb[cc][:, s,
                                                       oc * 128:(oc + 1) * 128],
                                        rhs=rhs,
                                        start=(cc == 0 and s == 0),
                                        stop=(cc == CC - 1 and s == 8))
                            xr = iopool.tile([128, 512], F32, tag="xr")
                            nc.sync.dma_start(out=xr,
                                              in_=xr_d[oc, :, t, nb])
                            ob = iopool.tile([128, 512], F32, tag="cob")
                            nc.vector.tensor_add(ob, ps, xr)
                            nc.sync.dma_start(out=out_d[oc, :, t, nb],
                                               in_=ob)

    nc.compile()
    return nc


_BUILD_CACHE = {}


def kernel(x, roi_inds, Wq, Wk, Wv, Wc, gn_gamma, gn_beta):
    global LAST_EXEC_NS
    x = np.ascontiguousarray(np.asarray(x, dtype=np.float32))
    roi_inds = np.asarray(roi_inds, dtype=np.int32)
    n, c = x.shape[0], x.shape[1]
    assert (n, c) == (N, C) and x.shape[2:] == (T, H, W)

    # group ROIs per clip; core g <- group g
    order = np.argsort(roi_inds, kind="stable")
    groups = [order[roi_inds[order] == g] for g in range(N_CORES)]
    sizes = [len(g) for g in groups]
    max_sz = max(sizes)
    assert max_sz <= NA, f"group size {max_sz} > {NA} unsupported"
    npad = ((max_sz + 7) // 8) * 8

    scale = 1.0 / np.sqrt(np.float32(C))

    def prep_w(Wt, sc=1.0):
        # [O, C, 1, 3, 3] -> [CC, 128, 9, O]
        w = (np.asarray(Wt, np.float32)[:, :, 0] * sc)  # [O, C, 3, 3]
        w = w.transpose(1, 2, 3, 0).reshape(CC, 128, 9, C)
        return np.ascontiguousarray(w).astype(ml_dtypes.bfloat16)

    w_arrs = {
        "wq": prep_w(Wq, scale), "wk": prep_w(Wk), "wv": prep_w(Wv),
        "wc": prep_w(Wc),
    }
    gamma = np.ascontiguousarray(np.asarray(gn_gamma, np.float32))
    beta = np.ascontiguousarray(np.asarray(gn_beta, np.float32))

    in_maps = []
    for g in range(N_CORES):
        idx = groups[g]
        xg = np.zeros((npad, C, T, H, W), np.float32)
        xg[:sizes[g]] = x[idx]
        # xp: [CC, 128, npad, T, 10, 10] zero-padded -> [CC,128,npad,T,100]
        xcm = xg.transpose(1, 0, 2, 3, 4).reshape(CC, 128, npad, T, H, W)
        xpp = np.zeros((CC, 128, npad, T, 10, 10), np.float32)
        xpp[..., 1:9, 1:9] = xcm
        # -> [CC, 128, NBK, T, 100, 8] with n innermost (contiguous for PE)
        xpp = xpp.reshape(CC, 128, npad // 8, 8, T, 100)
        xp = np.ascontiguousarray(
            xpp.transpose(0, 1, 2, 4, 5, 3)).astype(ml_dtypes.bfloat16)
        # xr: [CC, 128, T, NBK, (h,w,n8)] matching conv PSUM column order
        xr = np.ascontiguousarray(
            xcm.reshape(CC, 128, npad // 8, 8, T, HW)
            .transpose(0, 1, 4, 2, 5, 3)).reshape(
                CC, 128, T, npad // 8, 512)
        mask = np.zeros(NA, np.float32)
        mask[sizes[g]:] = -1e30
        m = {"xp": xp, "xr": xr, "mask": mask,
             "gamma": gamma, "beta": beta}
        m.update(w_arrs)
        in_maps.append(m)

    if npad not in _BUILD_CACHE:
        _BUILD_CACHE[npad] = _build(npad)
    nc = _BUILD_CACHE[npad]

    trace = bool(int(os.environ.get("ACAR_TRACE", "0")))
    if trace:
        _install_ntff_hook()
    res = run_bass_kernel_spmd(nc, in_maps, core_ids=list(range(N_CORES)),
                               trace=trace)
    LAST_EXEC_NS = res.exec_time_ns

    out = np.empty((N, C, T, H, W), np.float32)
    for g in range(N_CORES):
        og = res.results[g]["out"]  # [CC, 128, T, NBK, (hw, n8)]
        og = og.reshape(C, T, npad // 8, HW, 8)
        og = og.transpose(2, 4, 0, 1, 3).reshape(npad, C, T, H, W)
        out[groups[g]] = og[:sizes[g]]
    return out
